# revision 16
# baseline (speedup 1.0000x reference)
"""AspectMemoryEncoder Trainium2 kernel (8 NeuronCores, data-parallel over batch).

Reference computation (per batch row b):
    x = w_emb[ids[b, :]]                      # (S, E) embedding gather
    sim[s] = max_a cos(x[s], a_emb[a])        # aspect cosine-sim, max over aspects
    att = softmax_s(sim)                      # attention over sequence
    z = sum_s att[s] * x[s]                   # pooled output (E,)
Returns (z, att).

Distribution: batch (B=1024) sharded over 8 cores (128 rows each); the
embedding table and a_emb replicated.  No collectives.

Host-side weight repacking (properties of the weights alone): the table is
passed bf16-cast and padded as [V, 256] bf16 slots per row = [200 bf16
embedding][1 f32 inverse-norm (slots 200-201)][pad to 512B], so the gather
moves half the bytes of the f32 table in clean 512B descriptors and delivers
each token's eps-clamped inverse norm along with its embedding; the device
never computes norms.

Per-core layout: token columns c = h*128 + b (h = s-half, b = local batch
row); one column = 128 sequence positions s' on SBUF partitions.  Chunks of 16
columns are gathered with one indirect DMA (one 804B f32 row per token, cast
to bf16 on the SBUF write).  Per chunk:
  - transpose: PE transposes per column ([128,128] + [128,72] halves, E=200),
    quad-packed into one 2KB PSUM bank as bf16, copied to SBUF in two clean
    strips (weighted split between ACT and DVE)
  - sim: raw dots [128 s', 14] via matmuls with the transposed x as bf16
    stationary operand vs the normalized aspect matrix; reduce-max over
    aspects on DVE (max commutes with the positive per-token 1/norm scale,
    which is applied after via the gathered rsq column), then Exp
  - z: z^T columns accumulate in f32 PSUM with the bf16 x-chunk as stationary
    operand and the bf16 exp weights as moving operand
Only Copy/Exp/Square activation functions are used (all share table 0), so
there are no 1.28us activation-table reloads.  Cos-sim is in [-1,1] so the
softmax needs no max-subtraction; denominators come from a ones-vector matmul
over the bf16 exp matrix at the end, in f32 PSUM.
"""

import numpy as np

import concourse.bacc as bacc
import concourse.bass as bass
import concourse.tile as tile
from concourse import mybir
from concourse.bass_utils import run_bass_kernel_spmd
from concourse.masks import make_identity

N_CORES = 8
B, S, V, E, A = 1024, 256, 100000, 200, 14
EW = 256                   # stored row width in bf16 slots (512B padded rows)
RSQ_SLOT = 100             # f32 index of the inverse norm within a row
BC = B // N_CORES          # 128 batch rows per core
P = 128                    # SBUF partitions
H = S // P                 # 2 sequence halves
NCOL = BC * H              # 256 token columns per core, col c = h*128 + b
E1 = 128                   # E split for matmuls/transposes
E2 = E - E1                # 72
CHUNK = 16                 # token columns per gather chunk
NCHUNK = NCOL // CHUNK     # 16
QUAD = 4                   # columns per PSUM transpose bank

F32 = mybir.dt.float32
BF16 = mybir.dt.bfloat16
I32 = mybir.dt.int32
AF = mybir.ActivationFunctionType

# strip-copy engine split: ACT strips cost ~1.8x DVE strips (no 2x bf16 mode
# on ACT); DVE also owns the reduces, so ACT takes 2 of every 5 strips.
ACT_STRIPS = (0, 1)


def build_nc():
    nc = bacc.Bacc("TRN2", target_bir_lowering=False, debug=False,
                   num_devices=N_CORES)
    ids_d = nc.dram_tensor("ids", [P, NCOL], I32, kind="ExternalInput").ap()
    w_d = nc.dram_tensor("w_ext", [V, EW], BF16, kind="ExternalInput").ap()
    a_d = nc.dram_tensor("a_emb", [A, E], F32, kind="ExternalInput").ap()
    z_d = nc.dram_tensor("z_out", [BC, E], F32, kind="ExternalOutput").ap()
    att_d = nc.dram_tensor("att_out", [BC, S], F32, kind="ExternalOutput").ap()

    with tile.TileContext(nc) as tc:
        build_tile_kernel(tc, ids_d, w_d, a_d, z_d, att_d)
    nc.compile()
    return nc


def build_tile_kernel(tc, ids_d, w_d, a_d, z_d, att_d, dbg=None):
    nc = tc.nc
    from contextlib import ExitStack
    with ExitStack() as ctx:
        singles = ctx.enter_context(tc.tile_pool(name="singles", bufs=1))
        xgp = ctx.enter_context(tc.tile_pool(name="xg", bufs=3))
        xtp = ctx.enter_context(tc.tile_pool(name="xt", bufs=3))
        chp = ctx.enter_context(tc.tile_pool(name="chunk", bufs=3))
        ps_xt = ctx.enter_context(tc.tile_pool(name="ps_xt", bufs=3, space="PSUM"))
        ps_sim = ctx.enter_context(tc.tile_pool(name="ps_sim", bufs=2, space="PSUM"))
        ps_z = ctx.enter_context(tc.tile_pool(name="ps_z", bufs=1, space="PSUM"))
        ps_epi = ctx.enter_context(tc.tile_pool(name="ps_epi", bufs=2, space="PSUM"))

        # ---- constants ----
        ident = singles.tile([P, P], F32)
        make_identity(nc, ident[:])
        ident_bf = singles.tile([P, P], BF16)
        nc.vector.tensor_copy(out=ident_bf[:], in_=ident[:])
        ones_bf = singles.tile([P, 1], BF16)
        nc.vector.memset(ones_bf[:], 1.0)
        ids_sb = singles.tile([P, NCOL], I32)
        nc.sync.dma_start(out=ids_sb[:], in_=ids_d)

        # ---- normalize + transpose a_emb -> a_nT [E, A] (bf16, two chunks) ----
        a_sb = singles.tile([A, E], F32)
        nc.sync.dma_start(out=a_sb[:], in_=a_d)
        a_sq = singles.tile([A, E], F32)
        a_n2 = singles.tile([A, 1], F32)
        nc.scalar.activation(out=a_sq[:], in_=a_sb[:], func=AF.Square,
                             accum_out=a_n2[:])
        a_nrm = singles.tile([A, 1], F32)
        nc.scalar.activation(out=a_nrm[:], in_=a_n2[:], func=AF.Sqrt)
        a_rsq = singles.tile([A, 1], F32)
        nc.vector.reciprocal(out=a_rsq[:], in_=a_nrm[:])
        a_nn = singles.tile([A, E], F32)
        nc.vector.tensor_scalar_mul(a_nn[:], a_sb[:], a_rsq[:])
        anT_ps = ps_epi.tile([P, 2 * A], F32, tag="epi")
        nc.tensor.transpose(out=anT_ps[0:E1, 0:A], in_=a_nn[:, 0:E1],
                            identity=ident[0:A, 0:A])
        nc.tensor.transpose(out=anT_ps[0:E2, A:2 * A], in_=a_nn[:, E1:E],
                            identity=ident[0:A, 0:A])
        a_nT = singles.tile([P, 2 * A], BF16)
        nc.vector.tensor_copy(out=a_nT[:, 0:A], in_=anT_ps[:, 0:A])
        nc.vector.tensor_copy(out=a_nT[0:E2, A:2 * A], in_=anT_ps[0:E2, A:2 * A])

        # ---- persistent state ----
        expall = singles.tile([P, NCOL], BF16)  # exp(sim), col c = h*128+b
        zT = ps_z.tile([P, 2 * P], F32)         # z^T: cols 0:128 E1, 128:256 E2

        strip_i = [0]

        def strip_fn():
            i = strip_i[0]
            strip_i[0] = (i + 1) % 5
            if i in ACT_STRIPS:
                return lambda out, in_: nc.scalar.copy(out=out, in_=in_)
            return lambda out, in_: nc.vector.tensor_copy(out=out, in_=in_)

        # ---- main loop ----
        for ci in range(NCHUNK):
            c0 = ci * CHUNK
            xg = xgp.tile([P, CHUNK * EW], BF16)
            # HW indirect DMA consumes exactly one index per partition (the
            # sim's multi-index-per-partition view does not match silicon),
            # so gather 128 rows per instruction, one per column.
            for j in range(CHUNK):
                c = c0 + j
                nc.gpsimd.indirect_dma_start(
                    out=xg[:, j * EW:(j + 1) * EW], out_offset=None, in_=w_d,
                    in_offset=bass.IndirectOffsetOnAxis(
                        ap=ids_sb[:, c:c + 1], axis=0))

            # transpose per column into quad-packed bf16 PSUM banks:
            # quad q holds cols c0+4q..c0+4q+3; E1-halves at [128, 0:512],
            # E2-halves at [0:72, 512:1024]; two clean copy strips.
            xts = []
            for q in range(CHUNK // QUAD):
                pb = ps_xt.tile([P, 2 * QUAD * P], BF16)
                for u in range(QUAD):
                    j = q * QUAD + u
                    nc.tensor.transpose(
                        out=pb[:, u * P:(u + 1) * P],
                        in_=xg[:, j * EW:j * EW + E1],
                        identity=ident_bf[:])
                    nc.tensor.transpose(
                        out=pb[0:E2, QUAD * P + u * P:QUAD * P + (u + 1) * P],
                        in_=xg[:, j * EW + E1:j * EW + E],
                        identity=ident_bf[:])
                xt = xtp.tile([P, 2 * QUAD * P], BF16)
                strip_fn()(xt[:, 0:QUAD * P], pb[:, 0:QUAD * P])
                strip_fn()(xt[0:E2, QUAD * P:2 * QUAD * P],
                           pb[0:E2, QUAD * P:2 * QUAD * P])
                xts.append(xt)

            # sim: raw dots [s', A] per column, accumulated over E halves.
            # One accumulation group per ps_s bank (start only on the first
            # matmul; first touch writes, second accumulates).
            ps_s = ps_sim.tile([P, CHUNK * A], F32)
            for j in range(CHUNK):
                xt = xts[j // QUAD]
                u = j % QUAD
                nc.tensor.matmul(out=ps_s[:, j * A:(j + 1) * A],
                                 lhsT=xt[:, u * P:(u + 1) * P],
                                 rhs=a_nT[:, 0:A],
                                 start=(j == 0), stop=False,
                                 skip_group_check=True)
                nc.tensor.matmul(out=ps_s[:, j * A:(j + 1) * A],
                                 lhsT=xt[0:E2, QUAD * P + u * P:QUAD * P + (u + 1) * P],
                                 rhs=a_nT[0:E2, A:2 * A],
                                 start=False, stop=(j == CHUNK - 1),
                                 skip_group_check=True)

            # max over aspects, scale by gathered 1/norm, exponentiate
            mu = chp.tile([P, CHUNK], F32)
            nc.vector.tensor_reduce(
                out=mu[:],
                in_=ps_s[:].rearrange("p (m a) -> p m a", a=A),
                axis=mybir.AxisListType.X, op=mybir.AluOpType.max)
            rsqv = xg[:].bitcast(F32).rearrange(
                "p (j e) -> p j e", e=EW // 2)[:, :, RSQ_SLOT]
            simc = chp.tile([P, CHUNK], F32)
            nc.vector.tensor_mul(simc[:], mu[:], rsqv)
            nc.scalar.activation(out=expall[:, c0:c0 + CHUNK], in_=simc[:],
                                 func=AF.Exp)

            if dbg is not None and ci == 0:
                nc.sync.dma_start(out=dbg["xg"], in_=xg[:])
                nc.sync.dma_start(out=dbg["xt"], in_=xts[0][:, 0:QUAD * P])

            # z^T accumulation: x-chunk as stationary, exp col as moving.
            # One long accumulation group on the zT bank: only the very first
            # matmul carries start=True (whole-bank zero region); each
            # column's bytes are written at h=0 and accumulated at h=1.
            for j in range(CHUNK):
                c = c0 + j
                b = c % P
                nc.tensor.matmul(out=zT[:, b:b + 1],
                                 lhsT=xg[:, j * EW:j * EW + E1],
                                 rhs=expall[:, c:c + 1],
                                 start=(c == 0), stop=False,
                                 skip_group_check=True)
                nc.tensor.matmul(out=zT[0:E2, P + b:P + b + 1],
                                 lhsT=xg[:, j * EW + E1:j * EW + E],
                                 rhs=expall[:, c:c + 1],
                                 start=False, stop=(c == NCOL - 1),
                                 skip_group_check=True)

        # ---- epilogue: softmax denominators, outputs ----
        den_ps = ps_epi.tile([1, NCOL], F32, tag="epi")
        nc.tensor.matmul(out=den_ps[:], lhsT=ones_bf[:], rhs=expall[:],
                         start=True, stop=True)
        den_sb = singles.tile([1, NCOL], F32)
        nc.vector.tensor_copy(out=den_sb[:], in_=den_ps[:])
        den = singles.tile([1, P], F32)
        nc.vector.tensor_add(den[:], den_sb[0:1, 0:P], den_sb[0:1, P:NCOL])
        recip_row = singles.tile([1, P], F32)
        nc.vector.reciprocal(out=recip_row[:], in_=den[:])
        rT_ps = ps_epi.tile([P, 1], F32, tag="epi")
        nc.tensor.transpose(out=rT_ps[:], in_=recip_row[:],
                            identity=ident[0:1, 0:1])
        recip_col = singles.tile([P, 1], F32)
        nc.vector.tensor_copy(out=recip_col[:], in_=rT_ps[:])
        if dbg is not None:
            nc.sync.dma_start(out=dbg["exp"], in_=expall[:])
            nc.sync.dma_start(out=dbg["den"], in_=den[:])

        # attention output: transpose exp blocks to [b, s'] and scale
        att_sb = singles.tile([P, S], F32)
        for h in range(H):
            aT_ps = ps_epi.tile([P, P], BF16, tag="epi")
            nc.tensor.transpose(out=aT_ps[:], in_=expall[:, h * P:(h + 1) * P],
                                identity=ident_bf[:])
            nc.scalar.activation(out=att_sb[:, h * P:(h + 1) * P], in_=aT_ps[:],
                                 func=AF.Copy, scale=recip_col[:])
        nc.sync.dma_start(out=att_d, in_=att_sb[:])

        # z output: copy z^T to SBUF, transpose back to [b, E], scale
        zT_sb = singles.tile([P, 2 * P], F32)
        nc.vector.tensor_copy(out=zT_sb[:, 0:P], in_=zT[:, 0:P])
        nc.vector.tensor_copy(out=zT_sb[0:E2, P:2 * P], in_=zT[0:E2, P:2 * P])
        if dbg is not None:
            nc.sync.dma_start(out=dbg["zt"][0:P, 0:P], in_=zT_sb[:, 0:P])
            nc.sync.dma_start(out=dbg["zt"][0:E2, P:2 * P],
                              in_=zT_sb[0:E2, P:2 * P])
        z_ps = ps_epi.tile([P, E], F32, tag="epi")
        nc.tensor.transpose(out=z_ps[:, 0:E1], in_=zT_sb[:, 0:P],
                            identity=ident[:])
        nc.tensor.transpose(out=z_ps[:, E1:E], in_=zT_sb[0:E2, P:2 * P],
                            identity=ident[0:E2, 0:E2])
        z_sb = singles.tile([P, E], F32)
        nc.scalar.activation(out=z_sb[:], in_=z_ps[:], func=AF.Copy,
                             scale=recip_col[:])
        nc.sync.dma_start(out=z_d, in_=z_sb[:])


def make_in_maps(inputs, w_emb, a_emb):
    import ml_dtypes
    ids = np.asarray(inputs).astype(np.int32)
    w = np.asarray(w_emb, dtype=np.float32)
    ae = np.ascontiguousarray(np.asarray(a_emb, dtype=np.float32))
    # weight repacking: bf16 rows padded to 512B with the per-row f32
    # inverse norm (eps-clamped like the reference's cosine_similarity)
    rsq = (1.0 / np.maximum(np.linalg.norm(w.astype(np.float64), axis=1),
                            1e-8)).astype(np.float32)
    w_ext = np.zeros((V, EW), dtype=ml_dtypes.bfloat16)
    w_ext[:, :E] = w.astype(ml_dtypes.bfloat16)
    w_ext.view(np.float32)[:, RSQ_SLOT] = rsq
    in_maps = []
    for c in range(N_CORES):
        shard = ids[c * BC:(c + 1) * BC]                # (128, 256)
        u = shard.reshape(BC, H, P).transpose(2, 1, 0)  # (s', h, b)
        in_maps.append({
            "ids": np.ascontiguousarray(u.reshape(P, NCOL)),
            "w_ext": w_ext,
            "a_emb": ae,
        })
    return in_maps


_NC_CACHE = None


def get_nc():
    global _NC_CACHE
    if _NC_CACHE is None:
        _NC_CACHE = build_nc()
    return _NC_CACHE


def kernel(inputs, w_emb, a_emb):
    nc = get_nc()
    in_maps = make_in_maps(inputs, w_emb, a_emb)
    res = run_bass_kernel_spmd(nc, in_maps, core_ids=list(range(N_CORES)))
    z = np.concatenate([res.results[c]["z_out"] for c in range(N_CORES)], axis=0)
    a = np.concatenate([res.results[c]["att_out"] for c in range(N_CORES)], axis=0)
    return z, a


# revision 17
# speedup vs baseline: 119.8179x; 119.8179x over previous
"""AspectMemoryEncoder Trainium2 kernel (8 NeuronCores, data-parallel over batch).

Reference computation (per batch row b):
    x = w_emb[ids[b, :]]                      # (S, E) embedding gather
    sim[s] = max_a cos(x[s], a_emb[a])        # aspect cosine-sim, max over aspects
    att = softmax_s(sim)                      # attention over sequence
    z = sum_s att[s] * x[s]                   # pooled output (E,)
Returns (z, att).

Distribution: batch (B=1024) sharded over 8 cores (128 rows each); the
embedding table and a_emb replicated.  No collectives.

Host-side weight repacking (properties of the weights alone): the table is
passed bf16-cast and padded as [V, 256] bf16 slots per row = [200 bf16
embedding][1 f32 inverse-norm (slots 200-201)][pad to 512B], so the gather
moves half the bytes of the f32 table in clean 512B descriptors and delivers
each token's eps-clamped inverse norm along with its embedding; the device
never computes norms.

Per-core layout: token columns c = h*128 + b (h = s-half, b = local batch
row); one column = 128 sequence positions s' on SBUF partitions.  Chunks of 16
columns are gathered with one indirect DMA (one 804B f32 row per token, cast
to bf16 on the SBUF write).  Per chunk:
  - transpose: PE transposes per column ([128,128] + [128,72] halves, E=200),
    quad-packed into one 2KB PSUM bank as bf16, copied to SBUF in two clean
    strips (weighted split between ACT and DVE)
  - sim: raw dots [128 s', 14] via matmuls with the transposed x as bf16
    stationary operand vs the normalized aspect matrix; reduce-max over
    aspects on DVE (max commutes with the positive per-token 1/norm scale,
    which is applied after via the gathered rsq column), then Exp
  - z: z^T columns accumulate in f32 PSUM with the bf16 x-chunk as stationary
    operand and the bf16 exp weights as moving operand
Only Copy/Exp/Square activation functions are used (all share table 0), so
there are no 1.28us activation-table reloads.  Cos-sim is in [-1,1] so the
softmax needs no max-subtraction; denominators come from a ones-vector matmul
over the bf16 exp matrix at the end, in f32 PSUM.
"""

import numpy as np

import concourse.bacc as bacc
import concourse.bass as bass
import concourse.tile as tile
from concourse import mybir
from concourse.bass_utils import run_bass_kernel_spmd
from concourse.masks import make_identity

N_CORES = 8
B, S, V, E, A = 1024, 256, 100000, 200, 14
EW = 256                   # stored row width in bf16 slots (512B padded rows)
RSQ_SLOT = 100             # f32 index of the inverse norm within a row
BC = B // N_CORES          # 128 batch rows per core
P = 128                    # SBUF partitions
H = S // P                 # 2 sequence halves
NCOL = BC * H              # 256 token columns per core, col c = h*128 + b
E1 = 128                   # E split for matmuls/transposes
E2 = E - E1                # 72
CHUNK = 16                 # token columns per gather chunk
NCHUNK = NCOL // CHUNK     # 16
QUAD = 4                   # columns per PSUM transpose bank

F32 = mybir.dt.float32
BF16 = mybir.dt.bfloat16
I32 = mybir.dt.int32
AF = mybir.ActivationFunctionType

# strip-copy engine split: ACT strips cost ~1.8x DVE strips (no 2x bf16 mode
# on ACT); DVE also owns the reduces, so ACT takes 2 of every 5 strips.
ACT_STRIPS = (0, 1)


def build_nc(repeats=1):
    nc = bacc.Bacc("TRN2", target_bir_lowering=False, debug=False,
                   num_devices=N_CORES)
    ids_d = nc.dram_tensor("ids", [P, NCOL], I32, kind="ExternalInput").ap()
    w_d = nc.dram_tensor("w_ext", [V, EW], BF16, kind="ExternalInput").ap()
    a_d = nc.dram_tensor("a_emb", [A, E], F32, kind="ExternalInput").ap()
    z_d = nc.dram_tensor("z_out", [BC, E], F32, kind="ExternalOutput").ap()
    att_d = nc.dram_tensor("att_out", [BC, S], F32, kind="ExternalOutput").ap()

    with tile.TileContext(nc) as tc:
        build_tile_kernel(tc, ids_d, w_d, a_d, z_d, att_d, repeats=repeats)
    nc.compile()
    return nc


def build_tile_kernel(tc, ids_d, w_d, a_d, z_d, att_d, dbg=None, repeats=1):
    nc = tc.nc
    from contextlib import ExitStack
    with ExitStack() as ctx:
        singles = ctx.enter_context(tc.tile_pool(name="singles", bufs=1))
        xgp = ctx.enter_context(tc.tile_pool(name="xg", bufs=3))
        xtp = ctx.enter_context(tc.tile_pool(name="xt", bufs=3))
        chp = ctx.enter_context(tc.tile_pool(name="chunk", bufs=3))
        ps_xt = ctx.enter_context(tc.tile_pool(name="ps_xt", bufs=3, space="PSUM"))
        ps_sim = ctx.enter_context(tc.tile_pool(name="ps_sim", bufs=2, space="PSUM"))
        ps_z = ctx.enter_context(tc.tile_pool(name="ps_z", bufs=1, space="PSUM"))
        ps_epi = ctx.enter_context(tc.tile_pool(name="ps_epi", bufs=2, space="PSUM"))

        # ---- constants ----
        ident = singles.tile([P, P], F32)
        make_identity(nc, ident[:])
        ident_bf = singles.tile([P, P], BF16)
        nc.vector.tensor_copy(out=ident_bf[:], in_=ident[:])
        ones_bf = singles.tile([P, 1], BF16)
        nc.vector.memset(ones_bf[:], 1.0)
        ids_sb = singles.tile([P, NCOL], I32)
        nc.sync.dma_start(out=ids_sb[:], in_=ids_d)

        # ---- normalize + transpose a_emb -> a_nT [E, A] (bf16, two chunks) ----
        a_sb = singles.tile([A, E], F32)
        nc.sync.dma_start(out=a_sb[:], in_=a_d)
        a_sq = singles.tile([A, E], F32)
        a_n2 = singles.tile([A, 1], F32)
        nc.scalar.activation(out=a_sq[:], in_=a_sb[:], func=AF.Square,
                             accum_out=a_n2[:])
        a_nrm = singles.tile([A, 1], F32)
        nc.scalar.activation(out=a_nrm[:], in_=a_n2[:], func=AF.Sqrt)
        a_rsq = singles.tile([A, 1], F32)
        nc.vector.reciprocal(out=a_rsq[:], in_=a_nrm[:])
        a_nn = singles.tile([A, E], F32)
        nc.vector.tensor_scalar_mul(a_nn[:], a_sb[:], a_rsq[:])
        anT_ps = ps_epi.tile([P, 2 * A], F32, tag="epi")
        nc.tensor.transpose(out=anT_ps[0:E1, 0:A], in_=a_nn[:, 0:E1],
                            identity=ident[0:A, 0:A])
        nc.tensor.transpose(out=anT_ps[0:E2, A:2 * A], in_=a_nn[:, E1:E],
                            identity=ident[0:A, 0:A])
        a_nT = singles.tile([P, 2 * A], BF16)
        nc.vector.tensor_copy(out=a_nT[:, 0:A], in_=anT_ps[:, 0:A])
        nc.vector.tensor_copy(out=a_nT[0:E2, A:2 * A], in_=anT_ps[0:E2, A:2 * A])

        # ---- persistent state ----
        expall = singles.tile([P, NCOL], BF16)  # exp(sim), col c = h*128+b
        zT = ps_z.tile([P, 2 * P], F32)         # z^T: cols 0:128 E1, 128:256 E2

        strip_i = [0]

        def strip_fn():
            i = strip_i[0]
            strip_i[0] = (i + 1) % 5
            if i in ACT_STRIPS:
                return lambda out, in_: nc.scalar.copy(out=out, in_=in_)
            return lambda out, in_: nc.vector.tensor_copy(out=out, in_=in_)

        # ---- main loop (repeats > 1 only for timing variants: each pass
        # recomputes everything; the z accumulation restarts per pass) ----
        for rep in range(repeats):
          for ci in range(NCHUNK):
            c0 = ci * CHUNK
            xg = xgp.tile([P, CHUNK * EW], BF16)
            # HW indirect DMA consumes exactly one index per partition (the
            # sim's multi-index-per-partition view does not match silicon),
            # so gather 128 rows per instruction, one per column.
            for j in range(CHUNK):
                c = c0 + j
                nc.gpsimd.indirect_dma_start(
                    out=xg[:, j * EW:(j + 1) * EW], out_offset=None, in_=w_d,
                    in_offset=bass.IndirectOffsetOnAxis(
                        ap=ids_sb[:, c:c + 1], axis=0))

            # transpose per column into quad-packed bf16 PSUM banks:
            # quad q holds cols c0+4q..c0+4q+3; E1-halves at [128, 0:512],
            # E2-halves at [0:72, 512:1024]; two clean copy strips.
            xts = []
            for q in range(CHUNK // QUAD):
                pb = ps_xt.tile([P, 2 * QUAD * P], BF16)
                for u in range(QUAD):
                    j = q * QUAD + u
                    nc.tensor.transpose(
                        out=pb[:, u * P:(u + 1) * P],
                        in_=xg[:, j * EW:j * EW + E1],
                        identity=ident_bf[:])
                    nc.tensor.transpose(
                        out=pb[0:E2, QUAD * P + u * P:QUAD * P + (u + 1) * P],
                        in_=xg[:, j * EW + E1:j * EW + E],
                        identity=ident_bf[:])
                xt = xtp.tile([P, 2 * QUAD * P], BF16)
                strip_fn()(xt[:, 0:QUAD * P], pb[:, 0:QUAD * P])
                strip_fn()(xt[0:E2, QUAD * P:2 * QUAD * P],
                           pb[0:E2, QUAD * P:2 * QUAD * P])
                xts.append(xt)

            # sim: raw dots [s', A] per column, accumulated over E halves.
            # One accumulation group per ps_s bank (start only on the first
            # matmul; first touch writes, second accumulates).
            ps_s = ps_sim.tile([P, CHUNK * A], F32)
            for j in range(CHUNK):
                xt = xts[j // QUAD]
                u = j % QUAD
                nc.tensor.matmul(out=ps_s[:, j * A:(j + 1) * A],
                                 lhsT=xt[:, u * P:(u + 1) * P],
                                 rhs=a_nT[:, 0:A],
                                 start=(j == 0), stop=False,
                                 skip_group_check=True)
                nc.tensor.matmul(out=ps_s[:, j * A:(j + 1) * A],
                                 lhsT=xt[0:E2, QUAD * P + u * P:QUAD * P + (u + 1) * P],
                                 rhs=a_nT[0:E2, A:2 * A],
                                 start=False, stop=(j == CHUNK - 1),
                                 skip_group_check=True)

            # max over aspects, scale by gathered 1/norm, exponentiate
            mu = chp.tile([P, CHUNK], F32)
            nc.vector.tensor_reduce(
                out=mu[:],
                in_=ps_s[:].rearrange("p (m a) -> p m a", a=A),
                axis=mybir.AxisListType.X, op=mybir.AluOpType.max)
            rsqv = xg[:].bitcast(F32).rearrange(
                "p (j e) -> p j e", e=EW // 2)[:, :, RSQ_SLOT]
            simc = chp.tile([P, CHUNK], F32)
            nc.vector.tensor_mul(simc[:], mu[:], rsqv)
            nc.scalar.activation(out=expall[:, c0:c0 + CHUNK], in_=simc[:],
                                 func=AF.Exp)

            if dbg is not None and ci == 0:
                nc.sync.dma_start(out=dbg["xg"], in_=xg[:])
                nc.sync.dma_start(out=dbg["xt"], in_=xts[0][:, 0:QUAD * P])

            # z^T accumulation: x-chunk as stationary, exp col as moving.
            # One long accumulation group on the zT bank: only the very first
            # matmul carries start=True (whole-bank zero region); each
            # column's bytes are written at h=0 and accumulated at h=1.
            for j in range(CHUNK):
                c = c0 + j
                b = c % P
                nc.tensor.matmul(out=zT[:, b:b + 1],
                                 lhsT=xg[:, j * EW:j * EW + E1],
                                 rhs=expall[:, c:c + 1],
                                 start=(c == 0), stop=False,
                                 skip_group_check=True)
                nc.tensor.matmul(out=zT[0:E2, P + b:P + b + 1],
                                 lhsT=xg[:, j * EW + E1:j * EW + E],
                                 rhs=expall[:, c:c + 1],
                                 start=False, stop=(c == NCOL - 1),
                                 skip_group_check=True)

        # ---- epilogue: softmax denominators, outputs ----
        den_ps = ps_epi.tile([1, NCOL], F32, tag="epi")
        nc.tensor.matmul(out=den_ps[:], lhsT=ones_bf[:], rhs=expall[:],
                         start=True, stop=True)
        den_sb = singles.tile([1, NCOL], F32)
        nc.vector.tensor_copy(out=den_sb[:], in_=den_ps[:])
        den = singles.tile([1, P], F32)
        nc.vector.tensor_add(den[:], den_sb[0:1, 0:P], den_sb[0:1, P:NCOL])
        recip_row = singles.tile([1, P], F32)
        nc.vector.reciprocal(out=recip_row[:], in_=den[:])
        rT_ps = ps_epi.tile([P, 1], F32, tag="epi")
        nc.tensor.transpose(out=rT_ps[:], in_=recip_row[:],
                            identity=ident[0:1, 0:1])
        recip_col = singles.tile([P, 1], F32)
        nc.vector.tensor_copy(out=recip_col[:], in_=rT_ps[:])
        if dbg is not None:
            nc.sync.dma_start(out=dbg["exp"], in_=expall[:])
            nc.sync.dma_start(out=dbg["den"], in_=den[:])

        # attention output: transpose exp blocks to [b, s'] and scale
        att_sb = singles.tile([P, S], F32)
        for h in range(H):
            aT_ps = ps_epi.tile([P, P], BF16, tag="epi")
            nc.tensor.transpose(out=aT_ps[:], in_=expall[:, h * P:(h + 1) * P],
                                identity=ident_bf[:])
            nc.scalar.activation(out=att_sb[:, h * P:(h + 1) * P], in_=aT_ps[:],
                                 func=AF.Copy, scale=recip_col[:])
        nc.sync.dma_start(out=att_d, in_=att_sb[:])

        # z output: copy z^T to SBUF, transpose back to [b, E], scale
        zT_sb = singles.tile([P, 2 * P], F32)
        nc.vector.tensor_copy(out=zT_sb[:, 0:P], in_=zT[:, 0:P])
        nc.vector.tensor_copy(out=zT_sb[0:E2, P:2 * P], in_=zT[0:E2, P:2 * P])
        if dbg is not None:
            nc.sync.dma_start(out=dbg["zt"][0:P, 0:P], in_=zT_sb[:, 0:P])
            nc.sync.dma_start(out=dbg["zt"][0:E2, P:2 * P],
                              in_=zT_sb[0:E2, P:2 * P])
        z_ps = ps_epi.tile([P, E], F32, tag="epi")
        nc.tensor.transpose(out=z_ps[:, 0:E1], in_=zT_sb[:, 0:P],
                            identity=ident[:])
        nc.tensor.transpose(out=z_ps[:, E1:E], in_=zT_sb[0:E2, P:2 * P],
                            identity=ident[0:E2, 0:E2])
        z_sb = singles.tile([P, E], F32)
        nc.scalar.activation(out=z_sb[:], in_=z_ps[:], func=AF.Copy,
                             scale=recip_col[:])
        nc.sync.dma_start(out=z_d, in_=z_sb[:])


def make_in_maps(inputs, w_emb, a_emb):
    import ml_dtypes
    ids = np.asarray(inputs).astype(np.int32)
    w = np.asarray(w_emb, dtype=np.float32)
    ae = np.ascontiguousarray(np.asarray(a_emb, dtype=np.float32))
    # weight repacking: bf16 rows padded to 512B with the per-row f32
    # inverse norm (eps-clamped like the reference's cosine_similarity)
    rsq = (1.0 / np.maximum(np.linalg.norm(w.astype(np.float64), axis=1),
                            1e-8)).astype(np.float32)
    w_ext = np.zeros((V, EW), dtype=ml_dtypes.bfloat16)
    w_ext[:, :E] = w.astype(ml_dtypes.bfloat16)
    w_ext.view(np.float32)[:, RSQ_SLOT] = rsq
    in_maps = []
    for c in range(N_CORES):
        shard = ids[c * BC:(c + 1) * BC]                # (128, 256)
        u = shard.reshape(BC, H, P).transpose(2, 1, 0)  # (s', h, b)
        in_maps.append({
            "ids": np.ascontiguousarray(u.reshape(P, NCOL)),
            "w_ext": w_ext,
            "a_emb": ae,
        })
    return in_maps


_NC_CACHE = None


def get_nc():
    global _NC_CACHE
    if _NC_CACHE is None:
        _NC_CACHE = build_nc()
    return _NC_CACHE


def kernel(inputs, w_emb, a_emb):
    nc = get_nc()
    in_maps = make_in_maps(inputs, w_emb, a_emb)
    res = run_bass_kernel_spmd(nc, in_maps, core_ids=list(range(N_CORES)))
    z = np.concatenate([res.results[c]["z_out"] for c in range(N_CORES)], axis=0)
    a = np.concatenate([res.results[c]["att_out"] for c in range(N_CORES)], axis=0)
    return z, a
